# revision 14
# baseline (speedup 1.0000x reference)
"""Trainium2 Bass kernel for nn_CrossAttention (16x6209x256 cross-attention).

Strategy
--------
Data-parallel over batch: 16 batches -> 8 cores x 2 batches. Each core runs an
identical Bass/Tile program on its own batch slice (pure SPMD, no collectives).

Per batch the math is
    mapped_a = a @ Wa + ba            [6209, 64]
    mapped_b = b @ Wb + bb            [256, 64]
    scores   = mapped_a @ mapped_b.T * 8
    attn     = softmax(scores, -1)
    out      = (attn @ mapped_b) @ Wc + bc

With no nonlinearity between the projections and the attention matmuls, the
small weights fold per batch (computed on device in exact fp32):
    Wfused    = 8 * Wa @ mapped_b.T               [256, 256]
    scoreBias = 8 * ba @ mapped_b.T               [256]
    Wout      = mapped_b @ Wc + 1 x bc            [256, 256]
    scores    = a @ Wfused + scoreBias
    out       = softmax(scores) @ Wout        (bias bc exact since rows sum to 1)

Precision: softmax amplifies score error by |scores| (~500 here), so the
scores matmul runs as a 3-term bf16 split (a = ahi+alo split on host,
Wfused = Whi+Wlo split on device): scores ~ ahi@Whi + alo@Whi + ahi@Wlo,
residual ~1e-4 absolute. Downstream matmuls use f32r (1.4e-4 relative,
harmless there). Weight prep runs in exact fp32.

Layout: input_a is transposed on host to [256, seq] so the contraction dim
arrives on SBUF partitions straight from DMA; output is produced transposed
[256, seq] and transposed back on host. attn is normalized in [i, j] layout
on DVE (per-partition 1/sumexp), transposed to [j, i] via PE transpose.
"""
import sys

for _p in ("/opt/trn_rl_repo",):
    if _p not in sys.path:
        sys.path.append(_p)

import numpy as np
import ml_dtypes

import concourse.bacc as bacc
import concourse.mybir as mybir
import concourse.tile as tile
from concourse.bass_utils import run_bass_kernel_spmd

F32 = mybir.dt.float32
F32R = mybir.dt.float32r
BF16 = mybir.dt.bfloat16
P = 128

N_CORES = 8
BATCHES_PER_CORE = 2
SEQ = 6209
DF = 256          # feature dim of a / b
HID = 64          # projection dim
DMA_MACRO = 2048  # rows fetched/stored per DMA instruction
CMACRO = 512      # rows per compute macro (4 subtiles of 128)


def _row_plan(n_rows):
    """[(dma_start, dma_len, [(cm_start_within_dma, cm_len), ...]), ...]"""
    plan = []
    pos = 0
    while pos < n_rows:
        d = min(DMA_MACRO, n_rows - pos)
        cms = []
        q = 0
        while q < d:
            c = min(CMACRO, d - q)
            cms.append((q, c))
            q += c
        plan.append((pos, d, cms))
        pos += d
    return plan


def build_program(seq=SEQ, batches=BATCHES_PER_CORE, use_ba=False):
    nc = bacc.Bacc("TRN2", target_bir_lowering=False, debug=False)

    a_hl = nc.dram_tensor("a_hl", [batches, 2 * DF, seq], BF16, kind="ExternalInput")
    b_t = nc.dram_tensor("b_t", [batches, DF, DF], F32, kind="ExternalInput")
    wat = nc.dram_tensor("wat", [HID, DF], F32, kind="ExternalInput")
    wb = nc.dram_tensor("wb", [DF, HID], F32, kind="ExternalInput")
    wc = nc.dram_tensor("wc", [HID, DF], F32, kind="ExternalInput")
    ba_d = nc.dram_tensor("ba_d", [HID, 1], F32, kind="ExternalInput")
    bb_d = nc.dram_tensor("bb_d", [HID, 1], F32, kind="ExternalInput")
    bc_d = nc.dram_tensor("bc_d", [1, DF], F32, kind="ExternalInput")
    eye_d = nc.dram_tensor("eye_d", [P, P], F32, kind="ExternalInput")
    ones_d = nc.dram_tensor("ones_d", [1, P], F32, kind="ExternalInput")
    out_t = nc.dram_tensor("out_t", [batches, DF, seq], F32, kind="ExternalOutput")

    Exp = mybir.ActivationFunctionType.Exp
    Copy = mybir.ActivationFunctionType.Copy
    Ident = mybir.ActivationFunctionType.Identity

    with tile.TileContext(nc) as tc:
        with (
            tc.tile_pool(name="const", bufs=1) as cpool,
            tc.tile_pool(name="wpool", bufs=2) as wpool,
            tc.tile_pool(name="apool", bufs=3) as apool,
            tc.tile_pool(name="mpool", bufs=2) as mpool,
            tc.tile_pool(name="opool", bufs=3) as opool,
            tc.tile_pool(name="pp", bufs=1, space="PSUM") as pp,
        ):
            # ---- per-core constants ----
            eye_sb = cpool.tile([P, P], F32)
            nc.sync.dma_start(eye_sb[:], eye_d[:])
            wat_sb = cpool.tile([HID, DF], F32)
            nc.sync.dma_start(wat_sb[:], wat[:])
            wb_sb = cpool.tile([P, 2, HID], F32)
            nc.sync.dma_start(wb_sb[:], wb[:].rearrange("(k p) h -> p k h", p=P))
            wc_sb = cpool.tile([HID, DF], F32)
            nc.sync.dma_start(wc_sb[:], wc[:])
            ba_sb = cpool.tile([HID, 1], F32)
            nc.sync.dma_start(ba_sb[:], ba_d[:])
            bb_sb = cpool.tile([HID, 1], F32)
            nc.sync.dma_start(bb_sb[:], bb_d[:])
            bc_sb = cpool.tile([1, DF], F32)
            nc.sync.dma_start(bc_sb[:], bc_d[:])
            ones_sb = cpool.tile([1, P], F32)
            nc.sync.dma_start(ones_sb[:], ones_d[:])

            for b in range(batches):
                # ---- per-batch fused weights (exact fp32 matmuls) ----
                bT_sb = wpool.tile([P, 2, DF], F32)
                nc.sync.dma_start(bT_sb[:], b_t[b].rearrange("(k p) j -> p k j", p=P))

                ps_mb = pp.tile([HID, DF], F32, tag="fin0")
                for k in range(2):
                    nc.tensor.matmul(
                        ps_mb[:],
                        wb_sb[:, k, :],
                        bT_sb[:, k, :],
                        start=(k == 0), stop=(k == 1),
                    )
                mapped_bT = wpool.tile([HID, DF], F32)
                nc.scalar.activation(mapped_bT[:], ps_mb[:], Ident, bias=bb_sb[:])

                # Wfused, split hi/lo into bf16 (scale 8 folded in)
                whi_sb = wpool.tile([P, 2, DF], BF16)
                wlo_sb = wpool.tile([P, 2, DF], BF16)
                for c in range(2):
                    ps_wf = pp.tile([P, DF], F32, tag="fin0")
                    nc.tensor.matmul(
                        ps_wf[:],
                        wat_sb[:, c * P:(c + 1) * P],
                        mapped_bT[:],
                        start=True, stop=True,
                    )
                    nc.scalar.activation(whi_sb[:, c, :], ps_wf[:], Copy, scale=8.0)
                    # wlo = 8*wf - whi (rounded to bf16)
                    nc.vector.scalar_tensor_tensor(
                        wlo_sb[:, c, :],
                        ps_wf[:],
                        8.0,
                        whi_sb[:, c, :],
                        op0=mybir.AluOpType.mult,
                        op1=mybir.AluOpType.subtract,
                    )

                if use_ba:
                    ps_sbias = pp.tile([1, DF], F32, tag="fin0")
                    nc.tensor.matmul(
                        ps_sbias[:],
                        ba_sb[:],
                        mapped_bT[:],
                        start=True, stop=True,
                    )
                    sbias_sb = wpool.tile([1, DF], F32)
                    nc.scalar.activation(sbias_sb[:], ps_sbias[:], Copy, scale=8.0)

                wo_sb = wpool.tile([P, 2, DF], F32R)
                for c in range(2):
                    ps_wo = pp.tile([P, DF], F32, tag="fin0")
                    nc.tensor.matmul(
                        ps_wo[:],
                        mapped_bT[:, c * P:(c + 1) * P],
                        wc_sb[:],
                        start=True, stop=False,
                    )
                    nc.tensor.matmul(
                        ps_wo[:],
                        ones_sb[:],
                        bc_sb[:],
                        start=False, stop=True,
                    )
                    nc.vector.tensor_copy(wo_sb[:, c, :], ps_wo[:])

                # ---- main loop ----
                for d0, dlen, cms in _row_plan(seq):
                    aT_sb = apool.tile([P, 4, DMA_MACRO], BF16, tag="aT")
                    nc.sync.dma_start(
                        aT_sb[:, :, :dlen],
                        a_hl[b][:, d0:d0 + dlen].rearrange(
                            "(k p) i -> p k i", p=P),
                    )
                    outT_sb = opool.tile([P, 2, DMA_MACRO], F32, tag="outT")

                    for mo, R in cms:
                        subs = [(o, min(P, R - o)) for o in range(0, R, P)]
                        ns = len(subs)

                        scores_ps = pp.tile([P, 4 * DF], F32, tag="scores", bufs=2)
                        for s, (io, r) in enumerate(subs):
                            c0 = s * DF
                            terms = []
                            for k in range(2):
                                ah = aT_sb[:, k, mo + io:mo + io + r]
                                al = aT_sb[:, 2 + k, mo + io:mo + io + r]
                                terms += [
                                    (ah, whi_sb[:, k, :]),
                                    (al, whi_sb[:, k, :]),
                                    (ah, wlo_sb[:, k, :]),
                                ]
                            for t, (lhs, rhs) in enumerate(terms):
                                nc.tensor.matmul(
                                    scores_ps[:r, c0:c0 + DF],
                                    lhs,
                                    rhs,
                                    start=(t == 0),
                                    stop=(t == len(terms) - 1) and not use_ba,
                                )
                            if use_ba:
                                nc.tensor.matmul(
                                    scores_ps[:r, c0:c0 + DF],
                                    ones_sb[:, :r],
                                    sbias_sb[:],
                                    start=False, stop=True,
                                )

                        rmax = max(r for _, r in subs)
                        negmax = mpool.tile([P, 4], F32, tag="negmax")
                        if all(r == rmax for _, r in subs):
                            nc.vector.tensor_reduce(
                                negmax[:rmax, :ns],
                                scores_ps[:rmax, :ns * DF].rearrange(
                                    "p (s j) -> p s j", s=ns),
                                axis=mybir.AxisListType.X,
                                op=mybir.AluOpType.max,
                                negate=True,
                            )
                        else:
                            for s, (io, r) in enumerate(subs):
                                nc.vector.tensor_reduce(
                                    negmax[:r, s:s + 1],
                                    scores_ps[:r, s * DF:(s + 1) * DF],
                                    axis=mybir.AxisListType.X,
                                    op=mybir.AluOpType.max,
                                    negate=True,
                                )

                        attn_sb = mpool.tile([P, 4 * DF], F32, tag="attn")
                        attn_n = mpool.tile([P, 4 * DF], F32, tag="attn_n")
                        sumexp = mpool.tile([P, 4], F32, tag="sumexp")
                        for s, (io, r) in enumerate(subs):
                            c0 = s * DF
                            nc.scalar.activation(
                                attn_sb[:r, c0:c0 + DF],
                                scores_ps[:r, c0:c0 + DF],
                                Exp,
                                bias=negmax[:r, s:s + 1],
                                accum_out=sumexp[:r, s:s + 1],
                            )
                        recip = mpool.tile([P, 4], F32, tag="recip")
                        if all(r == rmax for _, r in subs):
                            nc.vector.reciprocal(recip[:rmax, :ns], sumexp[:rmax, :ns])
                        else:
                            for s, (io, r) in enumerate(subs):
                                nc.vector.reciprocal(
                                    recip[:r, s:s + 1], sumexp[:r, s:s + 1])
                        for s, (io, r) in enumerate(subs):
                            c0 = s * DF
                            nc.vector.tensor_scalar_mul(
                                attn_n[:r, c0:c0 + DF],
                                attn_sb[:r, c0:c0 + DF],
                                recip[:r, s:s + 1],
                            )

                        aT0_ps = pp.tile([P, CMACRO], F32, tag="attnT0")
                        aT1_ps = pp.tile([P, CMACRO], F32, tag="attnT1")
                        for s, (io, r) in enumerate(subs):
                            c0 = s * DF
                            for jh, dst in ((0, aT0_ps), (1, aT1_ps)):
                                o_ap = dst[:, io:io + r]
                                i_ap = attn_n[:r, c0 + jh * P:c0 + (jh + 1) * P]
                                e_ap = eye_sb[:r, :r]
                                if r % 2:
                                    # f32r transpose needs an even moving dim
                                    o_ap = o_ap.bitcast(F32)
                                    i_ap = i_ap.bitcast(F32)
                                    e_ap = e_ap.bitcast(F32)
                                nc.tensor.transpose(o_ap, i_ap, e_ap)
                        attnT0 = mpool.tile([P, CMACRO], F32R, tag="attnT0sb")
                        attnT1 = mpool.tile([P, CMACRO], F32R, tag="attnT1sb")
                        nc.scalar.copy(attnT0[:, :R], aT0_ps[:, :R])
                        nc.vector.tensor_copy(attnT1[:, :R], aT1_ps[:, :R])

                        # final: outT[fo, i] = sum_j Wout[j, fo] attnT[j, i]
                        for c in range(2):
                            ps_fin = pp.tile([P, CMACRO], F32, tag=f"fin{c}")
                            for k, aTk in enumerate((attnT0, attnT1)):
                                # f32r needs an even moving dim; odd tails
                                # fall back to plain fp32 (tiny anyway)
                                if R % 2 == 0:
                                    lhs, rhs = (wo_sb[:, k, c * P:(c + 1) * P],
                                                aTk[:, :R])
                                else:
                                    lhs = wo_sb[:, k, c * P:(c + 1) * P].bitcast(F32)
                                    rhs = aTk[:, :R].bitcast(F32)
                                nc.tensor.matmul(
                                    ps_fin[:, :R],
                                    lhs,
                                    rhs,
                                    start=(k == 0), stop=(k == 1),
                                )
                            if c == 0:
                                nc.vector.tensor_copy(
                                    outT_sb[:, c, mo:mo + R], ps_fin[:, :R])
                            else:
                                nc.scalar.copy(
                                    outT_sb[:, c, mo:mo + R], ps_fin[:, :R])

                    nc.sync.dma_start(
                        out_t[b][:, d0:d0 + dlen].rearrange("(c p) i -> p c i", p=P),
                        outT_sb[:, :, :dlen],
                    )

    nc.compile()
    return nc


_PROGRAM_CACHE = {}


def _get_program(seq=SEQ, batches=BATCHES_PER_CORE, use_ba=False):
    key = (seq, batches, use_ba)
    if key not in _PROGRAM_CACHE:
        _PROGRAM_CACHE[key] = build_program(seq, batches, use_ba)
    return _PROGRAM_CACHE[key]


def make_in_maps(input_a, input_b, Wa, ba, Wb, bb, Wc, bc,
                 n_cores=N_CORES, batches=BATCHES_PER_CORE):
    input_a = np.asarray(input_a, dtype=np.float32)
    input_b = np.asarray(input_b, dtype=np.float32)
    a_t = np.ascontiguousarray(input_a.transpose(0, 2, 1))      # [B, DF, seq]
    a_hi = a_t.astype(ml_dtypes.bfloat16)
    a_lo = (a_t - a_hi.astype(np.float32)).astype(ml_dtypes.bfloat16)
    # rows 0..DF-1 = hi, DF..2DF-1 = lo  -> [B, 2*DF, seq]
    a_hl = np.ascontiguousarray(np.concatenate([a_hi, a_lo], axis=1))
    b_t = np.ascontiguousarray(input_b.transpose(0, 2, 1))
    shared = {
        "wat": np.ascontiguousarray(np.asarray(Wa, np.float32).T),
        "wb": np.ascontiguousarray(np.asarray(Wb, np.float32)),
        "wc": np.ascontiguousarray(np.asarray(Wc, np.float32)),
        "ba_d": np.asarray(ba, np.float32).reshape(HID, 1).copy(),
        "bb_d": np.asarray(bb, np.float32).reshape(HID, 1).copy(),
        "bc_d": np.asarray(bc, np.float32).reshape(1, DF).copy(),
        "eye_d": np.eye(P, dtype=np.float32),
        "ones_d": np.ones((1, P), dtype=np.float32),
    }
    in_maps = []
    for c in range(n_cores):
        lo, hi = c * batches, (c + 1) * batches
        in_maps.append({
            "a_hl": np.ascontiguousarray(a_hl[lo:hi]),
            "b_t": np.ascontiguousarray(b_t[lo:hi]),
            **shared,
        })
    return in_maps


def kernel(input_a, input_b, Wa, ba, Wb, bb, Wc, bc):
    use_ba = bool(np.any(np.asarray(ba)))
    nc = _get_program(use_ba=use_ba)
    in_maps = make_in_maps(input_a, input_b, Wa, ba, Wb, bb, Wc, bc)
    res = run_bass_kernel_spmd(nc, in_maps, core_ids=list(range(N_CORES)))
    outs = np.concatenate([r["out_t"] for r in res.results], axis=0)
    return np.ascontiguousarray(outs.transpose(0, 2, 1))
